# revision 32
# baseline (speedup 1.0000x reference)
"""Trainium2 Bass kernel for nn_FIN_b: windowed-FM tabular net.

Data-parallel over batch: B=2048 rows split across 8 NeuronCores (256 each).

v2 design notes (vs the original baseline):
  * All weights/activations bf16 (PSUM accumulation stays f32); HBM traffic
    ~8.3 MB/core, 8 large DRAM->SBUF DMAs total, no SBUF->SBUF DMAs at all
    (the Tile scheduler was observed to drop DMA-completion waits for the
    shifted SBUF->SBUF copies, racing the DVE consumer).
  * Feature shifts for the FM block run on the tensor engine as matmuls with
    0/1 shift/selection matrices.
  * The channel dim is retiled into 9 overlapping 121-channel tiles
    (feature reach 121+7 <= 128), which removes the group-boundary straggler
    path entirely.
  * FM algebra (as in v1): fm[b,c] = sum_d sum_f D_d[b,c+f] G[c,f,f+d],
    D_d = x * shift_d(x), G[c,f,g] = sum_e v[c,f,e] v[c,g,e]; the FM linear
    term folds into W1's top half, b1 folds into a constant-1 row of fm.
  * W1 runs "flipped": stationary = activation blocks [128 feat, 128 batch],
    moving = W1 [128 feat, 512 hid]; h comes out batch-major, the final
    h @ W2 contraction runs on the vector engine (mul + free-dim reduce).
"""

import sys

sys.path.insert(0, "/opt/trn_rl_repo")

import numpy as np
import ml_dtypes

import concourse.bass as bass
import concourse.tile as tile
from concourse import bacc, mybir
from concourse.bass_utils import run_bass_kernel_spmd

NDF, NCF, NCC = 512, 256, 256
EMB, FIELD = 16, 8
B = 2048
NH0 = NDF + 2 * NCC          # 1024
CHANNEL = NH0 - FIELD + 1    # 1017
HID = (NH0 + CHANNEL) // 2   # 1020
NCORES = 8
BC = B // NCORES             # 256 batch rows per core
CT = 121                     # channels per overlapped tile (121 + 7 <= 128)
NT = 9                       # number of channel tiles (9*121 = 1089 >= 1017)

F32 = mybir.dt.float32
BF16 = mybir.dt.bfloat16

_cache = {}


def _sel_plan():
    """Selection-matrix plan for building overlapped tiles xo from x groups.

    Returns list over t of [(mat_idx, g), ...]: xo[:, t] = sum M_i^T x[:, g_i].
    t=0 is the identity on group 0 (handled by a DVE copy, no matmul).
    """
    plan = [[] for _ in range(NT)]
    idx = 0
    for t in range(1, NT):
        base = CT * t
        g0 = base // 128
        plan[t].append((idx, g0))
        idx += 1
        if g0 + 1 < 8 and (base + 127) // 128 > g0:
            plan[t].append((idx, g0 + 1))
            idx += 1
    return plan, idx


def _build(b2_val: float):
    nc = bacc.Bacc()

    plan, nsel = _sel_plan()
    NM = nsel + 7            # selection mats + shift mats S_1..S_7

    warm_d = nc.dram_tensor("warm", [128, 512], BF16, kind="ExternalInput")
    xdc_d = nc.dram_tensor("xdc", [128, 6, BC], BF16, kind="ExternalInput")
    Wda_d = nc.dram_tensor("Wda", [128, 4, NDF], BF16, kind="ExternalInput")
    Wdb_d = nc.dram_tensor("Wdb", [128, 2, NDF], BF16, kind="ExternalInput")
    bdc_d = nc.dram_tensor("bdc", [128, 8], F32, kind="ExternalInput")
    mats_d = nc.dram_tensor("mats", [128, NM, 128], BF16, kind="ExternalInput")
    oneh_d = nc.dram_tensor("oneh", [128, BC], BF16, kind="ExternalInput")
    Gm_d = nc.dram_tensor("Gm", [128, 7, NT, 128], BF16, kind="ExternalInput")
    W1a_d = nc.dram_tensor("W1a", [128, 8, 1024], BF16, kind="ExternalInput")
    W1b_d = nc.dram_tensor("W1b", [128, NT, 1024], BF16, kind="ExternalInput")
    w2b_d = nc.dram_tensor("w2b", [128, 1024], BF16, kind="ExternalInput")
    out_d = nc.dram_tensor("out", [128, 2], F32, kind="ExternalOutput")

    AF = mybir.ActivationFunctionType

    with tile.TileContext(nc) as tc:
        with (
            tc.tile_pool(name="w", bufs=1) as wp,
            tc.tile_pool(name="act", bufs=1) as ap,
            tc.tile_pool(name="pgen", bufs=1, space=bass.MemorySpace.PSUM) as pgen,
            tc.tile_pool(name="pfm", bufs=2, space=bass.MemorySpace.PSUM) as pfm,
            tc.tile_pool(name="pw1", bufs=1, space=bass.MemorySpace.PSUM) as pw1,
        ):
            # ---- weight/input DMAs, one issue per DRAM tensor, serialized
            # on sync in consumption order (parallel queues would make all
            # transfers stream concurrently and starve the front) ----
            warm = wp.tile([128, 512], BF16, tag="warm")
            nc.sync.dma_start(warm[:], warm_d[:])
            xdc = wp.tile([128, 6, BC], BF16, tag="xdc")
            nc.sync.dma_start(xdc[:], xdc_d[:])
            Wda = wp.tile([128, 4, NDF], BF16, tag="Wda")
            nc.sync.dma_start(Wda[:], Wda_d[:])
            bdc = wp.tile([128, 8], F32, tag="bdc")
            nc.sync.dma_start(bdc[:], bdc_d[:])
            Wdb = wp.tile([128, 2, NDF], BF16, tag="Wdb")
            nc.sync.dma_start(Wdb[:], Wdb_d[:])
            mats = wp.tile([128, NM, 128], BF16, tag="mats")
            nc.sync.dma_start(mats[:], mats_d[:])
            oneh = wp.tile([128, BC], BF16, tag="oneh")
            nc.sync.dma_start(oneh[:], oneh_d[:])
            W1a = wp.tile([128, 8, 1024], BF16, tag="W1a")
            nc.sync.dma_start(W1a[:], W1a_d[:])
            Gm = wp.tile([128, 7, NT, 128], BF16, tag="Gm")
            nc.sync.dma_start(Gm[:], Gm_d[:])
            w2b = wp.tile([128, 1024], BF16, tag="w2b")
            nc.sync.dma_start(w2b[:], w2b_d[:])
            W1b = wp.tile([128, NT, 1024], BF16, tag="W1b")
            nc.sync.dma_start(W1b[:], W1b_d[:])

            x = ap.tile([128, 8, BC], BF16, tag="x")
            xo = ap.tile([128, NT, BC], BF16, tag="xo")
            D = ap.tile([128, 7, NT, BC], BF16, tag="D")
            fm = ap.tile([128, NT, BC], BF16, tag="fm")
            h = ap.tile([128, 2, 1024], BF16, tag="h")
            hw0 = ap.tile([128, 1024], BF16, tag="hw0")
            hw1 = ap.tile([128, 1024], BF16, tag="hw1")
            hws = [hw0, hw1]
            sg = ap.tile([128, 2], F32, tag="sg")
            ot = ap.tile([128, 2], F32, tag="ot")
            # ---- PE p-state warmup: run junk matmuls into one PSUM slot
            # from when the first tiny DMA lands until the front weights
            # arrive, so the tensor engine is at full clock for real work ----
            psw = pfm.tile([128, BC], F32, tag="pf")
            for i in range(18):
                nc.tensor.matmul(
                    psw[:], warm[0:128, 0:128], warm[:, 0:BC],
                    start=True, stop=True,
                )

            # ---- front: x = relu([Xd|Xc] @ [Wd|Wc] + b), feature-major ----
            for mt in range(8):
                ps = pfm.tile([128, BC], F32, tag="pf")
                if mt < 4:
                    kts, W, col = [0, 1, 2, 3], Wda, mt * 128
                else:
                    kts, W, col = [0, 1], Wdb, (mt - 4) * 128
                for i, kt in enumerate(kts):
                    xg = kt if mt < 4 else 4 + kt
                    nc.tensor.matmul(
                        ps[:], W[:, kt, col:col + 128], xdc[:, xg, :],
                        start=(i == 0), stop=(i == len(kts) - 1),
                    )
                nc.scalar.activation(
                    out=x[:, mt, :], in_=ps[:], func=AF.Relu,
                    bias=bdc[:, mt:mt + 1], scale=1.0,
                )

            # ---- xo: overlapped 121-feature tiles (PE selection matmuls) ----
            nc.vector.tensor_copy(xo[:, 0, :], x[:, 0, :])
            for t in range(1, NT):
                ps = pfm.tile([128, BC], F32, tag="pf")
                ents = plan[t]
                for i, (mi, g) in enumerate(ents):
                    nc.tensor.matmul(
                        ps[:], mats[:, mi, :], x[:, g, :],
                        start=(i == 0), stop=(i == len(ents) - 1),
                    )
                nc.scalar.activation(
                    out=xo[:, t, :], in_=ps[:], func=AF.Copy, scale=1.0,
                )

            # ---- xs/D/FM software-pipelined rounds (t-major), with the
            # W1 x-part matmuls interleaved to fill tensor idle slots ----
            nsel0 = _sel_plan()[1]
            chunks = [(0, 3), (3, 3), (6, 3)]
            psW = []
            for q in range(4):
                psWq = pw1.tile([128, 512], F32, tag=f"pw{q}", name=f"psW{q}")
                psW.append(psWq)
            # W1 x-part work items in accumulation-safe order per psW tile
            w1x_items = [(bt, kt, n) for kt in range(8)
                         for bt in range(2) for n in range(2)]
            w1x_pos = 0

            def emit_w1x(count):
                nonlocal w1x_pos
                for _ in range(count):
                    if w1x_pos >= len(w1x_items):
                        return
                    bt, kt, n = w1x_items[w1x_pos]
                    nc.tensor.matmul(
                        psW[2 * bt + n][:],
                        x[:, kt, bt * 128:(bt + 1) * 128],
                        W1a[:, kt, n * 512:(n + 1) * 512],
                        start=(kt == 0), stop=False,
                    )
                    w1x_pos += 1

            def emit_fm_w1fm(j0, w):
                # FM accumulation for tiles [j0, j0+w), each followed by its
                # W1 fm-part matmuls (PSUM accumulation is order-independent;
                # only the start/stop flags must bracket the sequence)
                for t in range(j0, j0 + w):
                    ps = pfm.tile([128, BC], F32, tag="pf")
                    for d in range(1, 8):
                        nc.tensor.matmul(
                            ps[:], Gm[:, d - 1, t, :], D[:, d - 1, t, :],
                            start=(d == 1), stop=(d == 7),
                        )
                    if t < NT - 1:
                        nc.scalar.activation(
                            out=fm[:, t, :], in_=ps[:], func=AF.Copy,
                            scale=1.0,
                        )
                    else:
                        # adds the constant-1 row for the b1 fold (row 121)
                        nc.vector.tensor_add(fm[:, t, :], ps[:], oneh[:])
                    for bt in range(2):
                        for n in range(2):
                            nc.tensor.matmul(
                                psW[2 * bt + n][:],
                                fm[:, t, bt * 128:(bt + 1) * 128],
                                W1b[:, t, n * 512:(n + 1) * 512],
                                start=False, stop=(t == NT - 1),
                            )

            for k, (j0, w) in enumerate(chunks):
                for d in range(1, 8):
                    ps = pgen.tile([128, 768], F32, tag="pg")
                    nc.tensor.matmul(
                        ps[:, 0:2 * BC], mats[:, nsel0 + d - 1, :],
                        xo[:, j0:j0 + 2, :], start=True, stop=True,
                    )
                    nc.tensor.matmul(
                        ps[:, 2 * BC:3 * BC], mats[:, nsel0 + d - 1, :],
                        xo[:, j0 + 2, :], start=True, stop=True,
                    )
                    nc.vector.tensor_mul(
                        D[:, d - 1, j0:j0 + w, :], xo[:, j0:j0 + w, :],
                        ps[:, 0:w * BC],
                    )
                emit_w1x(11)
                if k > 0:
                    emit_fm_w1fm(*chunks[k - 1])
            emit_w1x(len(w1x_items))
            emit_fm_w1fm(*chunks[-1])

            # ---- lrelu + W2 (per batch-half, pipelined) ----
            for bt in range(2):
                for n in range(2):
                    nc.scalar.activation(
                        out=h[:, bt, n * 512:(n + 1) * 512],
                        in_=psW[2 * bt + n][:], func=AF.Lrelu,
                        scale=1.0, alpha=0.01,
                    )
                # W2 on DVE: sum_hid h*w2b
                nc.vector.tensor_mul(hws[bt][:], h[:, bt, :], w2b[:])
                nc.vector.tensor_reduce(
                    sg[:, bt:bt + 1], hws[bt][:],
                    axis=mybir.AxisListType.X, op=mybir.AluOpType.add,
                )
            nc.scalar.activation(
                out=ot[:], in_=sg[:], func=AF.Sigmoid, bias=b2_val, scale=1.0,
            )
            nc.sync.dma_start(out_d[:], ot[:])

    nc.finalize()
    return nc


def _prep_shared(inputs):
    """Host-side weight prep shared across cores (all bf16 on the wire)."""
    bf = ml_dtypes.bfloat16
    Wd = np.asarray(inputs["W_d"], np.float32)
    bd = np.asarray(inputs["b_d"], np.float32)
    Wc = np.asarray(inputs["W_c"], np.float32)
    bc = np.asarray(inputs["b_c"], np.float32)
    v = np.asarray(inputs["v"], np.float32)[0]          # [CHANNEL, FIELD, EMB]
    lin_w = np.asarray(inputs["lin_w"], np.float32)     # [FIELD, 1]
    lin_b = np.asarray(inputs["lin_b"], np.float32)     # [1]
    W1 = np.asarray(inputs["W1"], np.float32)           # [2041, HID]
    b1 = np.asarray(inputs["b1"], np.float32)
    W2 = np.asarray(inputs["W2"], np.float32)           # [HID, 1]

    # Wda/Wdb: stationary front blocks (Wd k-blocks / Wc k-blocks).
    Wda = np.zeros((128, 4, NDF), np.float32)
    for kt in range(4):
        Wda[:, kt, :] = Wd[kt * 128:(kt + 1) * 128, :]
    Wdb = np.zeros((128, 2, NDF), np.float32)
    for kt in range(2):
        Wdb[:, kt, :] = Wc[kt * 128:(kt + 1) * 128, :]
    bdc = np.concatenate([bd, bc]).reshape(8, 128).T.copy()  # [128, 8]

    # selection + shift matrices
    plan, nsel = _sel_plan()
    NM = nsel + 7
    mats = np.zeros((128, NM, 128), np.float32)
    for t in range(1, NT):
        base = CT * t
        for (mi, g) in plan[t]:
            for p in range(128):
                f = base + p
                if f >= NH0:
                    continue
                if f // 128 == g:
                    mats[f - 128 * g, mi, p] = 1.0
    for d in range(1, 8):
        for p in range(128 - d):
            mats[p + d, nsel + d - 1, p] = 1.0

    # banded FM weights on overlapped tiles:
    # Gm[p, d-1, t, m] = G[c=121t+m, f=p-m, f+d], 0<=f<=7-d, m<=120, c<CHANNEL
    G = np.einsum("cfe,cge->cfg", v, v)                 # [CHANNEL, 8, 8]
    Gm = np.zeros((128, 7, NT, 128), np.float32)
    for d in range(1, 8):
        for t in range(NT):
            for m in range(min(CT, CHANNEL - CT * t)):
                c = CT * t + m
                for f in range(0, 8 - d):
                    Gm[m + f, d - 1, t, m] = G[c, f, f + d]

    # fold the FM linear term (x_fm @ lin_w + lin_b) into W1's top half / b1
    W1a = W1[:NH0].copy()                               # [1024, HID]
    W1bfull = W1[NH0:]                                  # [CHANNEL, HID]
    for f in range(FIELD):
        W1a[f:f + CHANNEL, :] += lin_w[f, 0] * W1bfull
    b1e = b1 + lin_b[0] * W1bfull.sum(0)

    W1a_p = np.zeros((128, 8, 1024), np.float32)
    for kt in range(8):
        W1a_p[:, kt, :HID] = W1a[kt * 128:(kt + 1) * 128, :]
    # W1b on overlapped-tile rows; bias row at (t=8, p=121)
    W1b_p = np.zeros((128, NT, 1024), np.float32)
    for t in range(NT):
        for p in range(min(CT, max(0, CHANNEL - CT * t))):
            W1b_p[p, t, :HID] = W1bfull[CT * t + p, :]
    W1b_p[121, 8, :HID] = b1e

    w2b = np.zeros((128, 1024), np.float32)
    w2b[:, :HID] = W2[:, 0][None, :]

    oneh = np.zeros((128, BC), np.float32)
    oneh[121, :] = 1.0

    shared = {
        "warm": np.ones((128, 512), np.float32).astype(bf),
        "Wda": Wda.astype(bf),
        "Wdb": Wdb.astype(bf),
        "bdc": np.ascontiguousarray(bdc),
        "mats": mats.astype(bf),
        "oneh": oneh.astype(bf),
        "Gm": Gm.astype(bf),
        "W1a": W1a_p.astype(bf),
        "W1b": W1b_p.astype(bf),
        "w2b": w2b.astype(bf),
    }
    b2_val = float(np.asarray(inputs["b2"], np.float32)[0])
    return shared, b2_val


def _make_in_maps(inputs, shared):
    dx = np.asarray(inputs["discrete_x"], np.float32)   # [B, NDF]
    cx = np.asarray(inputs["continous_x"], np.float32)  # [B, NCF]
    bf = ml_dtypes.bfloat16
    in_maps = []
    for i in range(NCORES):
        dxi = dx[i * BC:(i + 1) * BC]                   # [BC, 512]
        cxi = cx[i * BC:(i + 1) * BC]                   # [BC, 256]
        xdc = np.empty((128, 6, BC), np.float32)
        for kt in range(4):
            xdc[:, kt, :] = dxi[:, kt * 128:(kt + 1) * 128].T
        for kt in range(2):
            xdc[:, 4 + kt, :] = cxi[:, kt * 128:(kt + 1) * 128].T
        m = dict(shared)
        m["xdc"] = xdc.astype(bf)
        in_maps.append(m)
    return in_maps


def kernel(**inputs) -> np.ndarray:
    shared, b2_val = _prep_shared(inputs)

    if "nc" not in _cache or _cache.get("b2") != b2_val:
        _cache["nc"] = _build(b2_val)
        _cache["b2"] = b2_val
    nc = _cache["nc"]

    in_maps = _make_in_maps(inputs, shared)
    res = run_bass_kernel_spmd(nc, in_maps, core_ids=list(range(NCORES)))
    out = np.empty((B, 1), np.float32)
    for i in range(NCORES):
        o = res.results[i]["out"]                       # [128, 2]
        out[i * BC:i * BC + 128, 0] = o[:, 0]
        out[i * BC + 128:(i + 1) * BC, 0] = o[:, 1]
    return out


# revision 33
# speedup vs baseline: 1.0607x; 1.0607x over previous
"""Trainium2 Bass kernel for nn_FIN_b: windowed-FM tabular net.

Data-parallel over batch: B=2048 rows split across 8 NeuronCores (256 each).

v2 design notes (vs the original baseline):
  * All weights/activations bf16 (PSUM accumulation stays f32); HBM traffic
    ~8.3 MB/core, 8 large DRAM->SBUF DMAs total, no SBUF->SBUF DMAs at all
    (the Tile scheduler was observed to drop DMA-completion waits for the
    shifted SBUF->SBUF copies, racing the DVE consumer).
  * Feature shifts for the FM block run on the tensor engine as matmuls with
    0/1 shift/selection matrices.
  * The channel dim is retiled into 9 overlapping 121-channel tiles
    (feature reach 121+7 <= 128), which removes the group-boundary straggler
    path entirely.
  * FM algebra (as in v1): fm[b,c] = sum_d sum_f D_d[b,c+f] G[c,f,f+d],
    D_d = x * shift_d(x), G[c,f,g] = sum_e v[c,f,e] v[c,g,e]; the FM linear
    term folds into W1's top half, b1 folds into a constant-1 row of fm.
  * W1 runs "flipped": stationary = activation blocks [128 feat, 128 batch],
    moving = W1 [128 feat, 512 hid]; h comes out batch-major, the final
    h @ W2 contraction runs on the vector engine (mul + free-dim reduce).
"""

import sys

sys.path.insert(0, "/opt/trn_rl_repo")

import numpy as np
import ml_dtypes

import concourse.bass as bass
import concourse.tile as tile
from concourse import bacc, mybir
from concourse.bass_utils import run_bass_kernel_spmd

NDF, NCF, NCC = 512, 256, 256
EMB, FIELD = 16, 8
B = 2048
NH0 = NDF + 2 * NCC          # 1024
CHANNEL = NH0 - FIELD + 1    # 1017
HID = (NH0 + CHANNEL) // 2   # 1020
NCORES = 8
BC = B // NCORES             # 256 batch rows per core
CT = 121                     # channels per overlapped tile (121 + 7 <= 128)
NT = 9                       # number of channel tiles (9*121 = 1089 >= 1017)

F32 = mybir.dt.float32
BF16 = mybir.dt.bfloat16

_cache = {}


def _sel_plan():
    """Selection-matrix plan for building overlapped tiles xo from x groups.

    Returns list over t of [(mat_idx, g), ...]: xo[:, t] = sum M_i^T x[:, g_i].
    t=0 is the identity on group 0 (handled by a DVE copy, no matmul).
    """
    plan = [[] for _ in range(NT)]
    idx = 0
    for t in range(1, NT):
        base = CT * t
        g0 = base // 128
        plan[t].append((idx, g0))
        idx += 1
        if g0 + 1 < 8 and (base + 127) // 128 > g0:
            plan[t].append((idx, g0 + 1))
            idx += 1
    return plan, idx


def _build(b2_val: float):
    nc = bacc.Bacc()

    plan, nsel = _sel_plan()
    NM = nsel + 7            # selection mats + shift mats S_1..S_7

    warm_d = nc.dram_tensor("warm", [128, 512], BF16, kind="ExternalInput")
    xdc_d = nc.dram_tensor("xdc", [128, 6, BC], BF16, kind="ExternalInput")
    Wda_d = nc.dram_tensor("Wda", [128, 4, NDF], BF16, kind="ExternalInput")
    Wdb_d = nc.dram_tensor("Wdb", [128, 2, NDF], BF16, kind="ExternalInput")
    bdc_d = nc.dram_tensor("bdc", [128, 8], F32, kind="ExternalInput")
    mats_d = nc.dram_tensor("mats", [128, NM, 128], BF16, kind="ExternalInput")
    oneh_d = nc.dram_tensor("oneh", [128, BC], BF16, kind="ExternalInput")
    Gm_d = nc.dram_tensor("Gm", [128, 7, NT, 128], BF16, kind="ExternalInput")
    W1a_d = nc.dram_tensor("W1a", [128, 8, 1024], BF16, kind="ExternalInput")
    W1b_d = nc.dram_tensor("W1b", [128, NT, 1024], BF16, kind="ExternalInput")
    w2b_d = nc.dram_tensor("w2b", [128, 1024], BF16, kind="ExternalInput")
    out_d = nc.dram_tensor("out", [128, 2], F32, kind="ExternalOutput")

    AF = mybir.ActivationFunctionType

    with tile.TileContext(nc) as tc:
        with (
            tc.tile_pool(name="w", bufs=1) as wp,
            tc.tile_pool(name="act", bufs=1) as ap,
            tc.tile_pool(name="pgen", bufs=2, space=bass.MemorySpace.PSUM) as pgen,
            tc.tile_pool(name="pfm", bufs=2, space=bass.MemorySpace.PSUM) as pfm,
            tc.tile_pool(name="pw1", bufs=1, space=bass.MemorySpace.PSUM) as pw1,
        ):
            # ---- weight/input DMAs, one issue per DRAM tensor, serialized
            # on sync in consumption order (parallel queues would make all
            # transfers stream concurrently and starve the front) ----
            warm = wp.tile([128, 512], BF16, tag="warm")
            nc.sync.dma_start(warm[:], warm_d[:])
            xdc = wp.tile([128, 6, BC], BF16, tag="xdc")
            nc.sync.dma_start(xdc[:], xdc_d[:])
            Wda = wp.tile([128, 4, NDF], BF16, tag="Wda")
            nc.sync.dma_start(Wda[:], Wda_d[:])
            bdc = wp.tile([128, 8], F32, tag="bdc")
            nc.sync.dma_start(bdc[:], bdc_d[:])
            Wdb = wp.tile([128, 2, NDF], BF16, tag="Wdb")
            nc.sync.dma_start(Wdb[:], Wdb_d[:])
            mats = wp.tile([128, NM, 128], BF16, tag="mats")
            nc.sync.dma_start(mats[:], mats_d[:])
            oneh = wp.tile([128, BC], BF16, tag="oneh")
            nc.sync.dma_start(oneh[:], oneh_d[:])
            W1a = wp.tile([128, 8, 1024], BF16, tag="W1a")
            nc.sync.dma_start(W1a[:], W1a_d[:])
            Gm = wp.tile([128, 7, NT, 128], BF16, tag="Gm")
            nc.sync.dma_start(Gm[:], Gm_d[:])
            w2b = wp.tile([128, 1024], BF16, tag="w2b")
            nc.sync.dma_start(w2b[:], w2b_d[:])
            W1b = wp.tile([128, NT, 1024], BF16, tag="W1b")
            nc.sync.dma_start(W1b[:], W1b_d[:])

            x = ap.tile([128, 8, BC], BF16, tag="x")
            xo = ap.tile([128, NT, BC], BF16, tag="xo")
            D = ap.tile([128, 7, NT, BC], BF16, tag="D")
            fm = ap.tile([128, NT, BC], BF16, tag="fm")
            h = ap.tile([128, 2, 1024], BF16, tag="h")
            hw0 = ap.tile([128, 1024], BF16, tag="hw0")
            hw1 = ap.tile([128, 1024], BF16, tag="hw1")
            hws = [hw0, hw1]
            sg = ap.tile([128, 2], F32, tag="sg")
            ot = ap.tile([128, 2], F32, tag="ot")
            # ---- PE p-state warmup: run junk matmuls into one PSUM slot
            # from when the first tiny DMA lands until the front weights
            # arrive, so the tensor engine is at full clock for real work ----
            psw = pfm.tile([128, BC], F32, tag="pf")
            for i in range(18):
                nc.tensor.matmul(
                    psw[:], warm[0:128, 0:128], warm[:, 0:BC],
                    start=True, stop=True,
                )

            # ---- front: x = relu([Xd|Xc] @ [Wd|Wc] + b), feature-major ----
            for mt in range(8):
                ps = pfm.tile([128, BC], F32, tag="pf")
                if mt < 4:
                    kts, W, col = [0, 1, 2, 3], Wda, mt * 128
                else:
                    kts, W, col = [0, 1], Wdb, (mt - 4) * 128
                for i, kt in enumerate(kts):
                    xg = kt if mt < 4 else 4 + kt
                    nc.tensor.matmul(
                        ps[:], W[:, kt, col:col + 128], xdc[:, xg, :],
                        start=(i == 0), stop=(i == len(kts) - 1),
                    )
                nc.scalar.activation(
                    out=x[:, mt, :], in_=ps[:], func=AF.Relu,
                    bias=bdc[:, mt:mt + 1], scale=1.0,
                )

            # ---- xo: overlapped 121-feature tiles (PE selection matmuls) ----
            nc.vector.tensor_copy(xo[:, 0, :], x[:, 0, :])
            for t in range(1, NT):
                ps = pfm.tile([128, BC], F32, tag="pf")
                ents = plan[t]
                for i, (mi, g) in enumerate(ents):
                    nc.tensor.matmul(
                        ps[:], mats[:, mi, :], x[:, g, :],
                        start=(i == 0), stop=(i == len(ents) - 1),
                    )
                nc.scalar.activation(
                    out=xo[:, t, :], in_=ps[:], func=AF.Copy, scale=1.0,
                )

            # ---- xs/D/FM software-pipelined rounds (t-major), with the
            # W1 x-part matmuls interleaved to fill tensor idle slots ----
            nsel0 = _sel_plan()[1]
            chunks = [(0, 2), (2, 2), (4, 2), (6, 2), (8, 1)]
            psW = []
            for q in range(4):
                psWq = pw1.tile([128, 512], F32, tag=f"pw{q}", name=f"psW{q}")
                psW.append(psWq)
            # W1 x-part work items in accumulation-safe order per psW tile
            w1x_items = [(bt, kt, n) for kt in range(8)
                         for bt in range(2) for n in range(2)]
            w1x_pos = 0

            def emit_w1x(count):
                nonlocal w1x_pos
                for _ in range(count):
                    if w1x_pos >= len(w1x_items):
                        return
                    bt, kt, n = w1x_items[w1x_pos]
                    nc.tensor.matmul(
                        psW[2 * bt + n][:],
                        x[:, kt, bt * 128:(bt + 1) * 128],
                        W1a[:, kt, n * 512:(n + 1) * 512],
                        start=(kt == 0), stop=False,
                    )
                    w1x_pos += 1

            def emit_fm_w1fm(j0, w):
                # FM accumulation for tiles [j0, j0+w), each followed by its
                # W1 fm-part matmuls (PSUM accumulation is order-independent;
                # only the start/stop flags must bracket the sequence)
                for t in range(j0, j0 + w):
                    ps = pfm.tile([128, BC], F32, tag="pf")
                    for d in range(1, 8):
                        nc.tensor.matmul(
                            ps[:], Gm[:, d - 1, t, :], D[:, d - 1, t, :],
                            start=(d == 1), stop=(d == 7),
                        )
                    if t < NT - 1:
                        nc.scalar.activation(
                            out=fm[:, t, :], in_=ps[:], func=AF.Copy,
                            scale=1.0,
                        )
                    else:
                        # adds the constant-1 row for the b1 fold (row 121)
                        nc.vector.tensor_add(fm[:, t, :], ps[:], oneh[:])
                    for bt in range(2):
                        for n in range(2):
                            nc.tensor.matmul(
                                psW[2 * bt + n][:],
                                fm[:, t, bt * 128:(bt + 1) * 128],
                                W1b[:, t, n * 512:(n + 1) * 512],
                                start=False, stop=(t == NT - 1),
                            )

            for k, (j0, w) in enumerate(chunks):
                for d in range(1, 8):
                    ps = pgen.tile([128, 512], F32, tag="pg")
                    nc.tensor.matmul(
                        ps[:, 0:w * BC], mats[:, nsel0 + d - 1, :],
                        xo[:, j0:j0 + w, :], start=True, stop=True,
                    )
                    nc.vector.tensor_mul(
                        D[:, d - 1, j0:j0 + w, :], xo[:, j0:j0 + w, :],
                        ps[:, 0:w * BC],
                    )
                emit_w1x(7)
                if k > 0:
                    emit_fm_w1fm(*chunks[k - 1])
            emit_w1x(len(w1x_items))
            emit_fm_w1fm(*chunks[-1])

            # ---- lrelu + W2 (per batch-half, pipelined) ----
            for bt in range(2):
                for n in range(2):
                    nc.scalar.activation(
                        out=h[:, bt, n * 512:(n + 1) * 512],
                        in_=psW[2 * bt + n][:], func=AF.Lrelu,
                        scale=1.0, alpha=0.01,
                    )
                # W2 on DVE: sum_hid h*w2b
                nc.vector.tensor_mul(hws[bt][:], h[:, bt, :], w2b[:])
                nc.vector.tensor_reduce(
                    sg[:, bt:bt + 1], hws[bt][:],
                    axis=mybir.AxisListType.X, op=mybir.AluOpType.add,
                )
            nc.scalar.activation(
                out=ot[:], in_=sg[:], func=AF.Sigmoid, bias=b2_val, scale=1.0,
            )
            nc.sync.dma_start(out_d[:], ot[:])

    nc.finalize()
    return nc


def _prep_shared(inputs):
    """Host-side weight prep shared across cores (all bf16 on the wire)."""
    bf = ml_dtypes.bfloat16
    Wd = np.asarray(inputs["W_d"], np.float32)
    bd = np.asarray(inputs["b_d"], np.float32)
    Wc = np.asarray(inputs["W_c"], np.float32)
    bc = np.asarray(inputs["b_c"], np.float32)
    v = np.asarray(inputs["v"], np.float32)[0]          # [CHANNEL, FIELD, EMB]
    lin_w = np.asarray(inputs["lin_w"], np.float32)     # [FIELD, 1]
    lin_b = np.asarray(inputs["lin_b"], np.float32)     # [1]
    W1 = np.asarray(inputs["W1"], np.float32)           # [2041, HID]
    b1 = np.asarray(inputs["b1"], np.float32)
    W2 = np.asarray(inputs["W2"], np.float32)           # [HID, 1]

    # Wda/Wdb: stationary front blocks (Wd k-blocks / Wc k-blocks).
    Wda = np.zeros((128, 4, NDF), np.float32)
    for kt in range(4):
        Wda[:, kt, :] = Wd[kt * 128:(kt + 1) * 128, :]
    Wdb = np.zeros((128, 2, NDF), np.float32)
    for kt in range(2):
        Wdb[:, kt, :] = Wc[kt * 128:(kt + 1) * 128, :]
    bdc = np.concatenate([bd, bc]).reshape(8, 128).T.copy()  # [128, 8]

    # selection + shift matrices
    plan, nsel = _sel_plan()
    NM = nsel + 7
    mats = np.zeros((128, NM, 128), np.float32)
    for t in range(1, NT):
        base = CT * t
        for (mi, g) in plan[t]:
            for p in range(128):
                f = base + p
                if f >= NH0:
                    continue
                if f // 128 == g:
                    mats[f - 128 * g, mi, p] = 1.0
    for d in range(1, 8):
        for p in range(128 - d):
            mats[p + d, nsel + d - 1, p] = 1.0

    # banded FM weights on overlapped tiles:
    # Gm[p, d-1, t, m] = G[c=121t+m, f=p-m, f+d], 0<=f<=7-d, m<=120, c<CHANNEL
    G = np.einsum("cfe,cge->cfg", v, v)                 # [CHANNEL, 8, 8]
    Gm = np.zeros((128, 7, NT, 128), np.float32)
    for d in range(1, 8):
        for t in range(NT):
            for m in range(min(CT, CHANNEL - CT * t)):
                c = CT * t + m
                for f in range(0, 8 - d):
                    Gm[m + f, d - 1, t, m] = G[c, f, f + d]

    # fold the FM linear term (x_fm @ lin_w + lin_b) into W1's top half / b1
    W1a = W1[:NH0].copy()                               # [1024, HID]
    W1bfull = W1[NH0:]                                  # [CHANNEL, HID]
    for f in range(FIELD):
        W1a[f:f + CHANNEL, :] += lin_w[f, 0] * W1bfull
    b1e = b1 + lin_b[0] * W1bfull.sum(0)

    W1a_p = np.zeros((128, 8, 1024), np.float32)
    for kt in range(8):
        W1a_p[:, kt, :HID] = W1a[kt * 128:(kt + 1) * 128, :]
    # W1b on overlapped-tile rows; bias row at (t=8, p=121)
    W1b_p = np.zeros((128, NT, 1024), np.float32)
    for t in range(NT):
        for p in range(min(CT, max(0, CHANNEL - CT * t))):
            W1b_p[p, t, :HID] = W1bfull[CT * t + p, :]
    W1b_p[121, 8, :HID] = b1e

    w2b = np.zeros((128, 1024), np.float32)
    w2b[:, :HID] = W2[:, 0][None, :]

    oneh = np.zeros((128, BC), np.float32)
    oneh[121, :] = 1.0

    shared = {
        "warm": np.ones((128, 512), np.float32).astype(bf),
        "Wda": Wda.astype(bf),
        "Wdb": Wdb.astype(bf),
        "bdc": np.ascontiguousarray(bdc),
        "mats": mats.astype(bf),
        "oneh": oneh.astype(bf),
        "Gm": Gm.astype(bf),
        "W1a": W1a_p.astype(bf),
        "W1b": W1b_p.astype(bf),
        "w2b": w2b.astype(bf),
    }
    b2_val = float(np.asarray(inputs["b2"], np.float32)[0])
    return shared, b2_val


def _make_in_maps(inputs, shared):
    dx = np.asarray(inputs["discrete_x"], np.float32)   # [B, NDF]
    cx = np.asarray(inputs["continous_x"], np.float32)  # [B, NCF]
    bf = ml_dtypes.bfloat16
    in_maps = []
    for i in range(NCORES):
        dxi = dx[i * BC:(i + 1) * BC]                   # [BC, 512]
        cxi = cx[i * BC:(i + 1) * BC]                   # [BC, 256]
        xdc = np.empty((128, 6, BC), np.float32)
        for kt in range(4):
            xdc[:, kt, :] = dxi[:, kt * 128:(kt + 1) * 128].T
        for kt in range(2):
            xdc[:, 4 + kt, :] = cxi[:, kt * 128:(kt + 1) * 128].T
        m = dict(shared)
        m["xdc"] = xdc.astype(bf)
        in_maps.append(m)
    return in_maps


def kernel(**inputs) -> np.ndarray:
    shared, b2_val = _prep_shared(inputs)

    if "nc" not in _cache or _cache.get("b2") != b2_val:
        _cache["nc"] = _build(b2_val)
        _cache["b2"] = b2_val
    nc = _cache["nc"]

    in_maps = _make_in_maps(inputs, shared)
    res = run_bass_kernel_spmd(nc, in_maps, core_ids=list(range(NCORES)))
    out = np.empty((B, 1), np.float32)
    for i in range(NCORES):
        o = res.results[i]["out"]                       # [128, 2]
        out[i * BC:i * BC + 128, 0] = o[:, 0]
        out[i * BC + 128:(i + 1) * BC, 0] = o[:, 1]
    return out


# revision 34
# speedup vs baseline: 1.1283x; 1.0637x over previous
"""Trainium2 Bass kernel for nn_FIN_b: windowed-FM tabular net.

Data-parallel over batch: B=2048 rows split across 8 NeuronCores (256 each).

v2 design notes (vs the original baseline):
  * All weights/activations bf16 (PSUM accumulation stays f32); HBM traffic
    ~8.3 MB/core, 8 large DRAM->SBUF DMAs total, no SBUF->SBUF DMAs at all
    (the Tile scheduler was observed to drop DMA-completion waits for the
    shifted SBUF->SBUF copies, racing the DVE consumer).
  * Feature shifts for the FM block run on the tensor engine as matmuls with
    0/1 shift/selection matrices.
  * The channel dim is retiled into 9 overlapping 121-channel tiles
    (feature reach 121+7 <= 128), which removes the group-boundary straggler
    path entirely.
  * FM algebra (as in v1): fm[b,c] = sum_d sum_f D_d[b,c+f] G[c,f,f+d],
    D_d = x * shift_d(x), G[c,f,g] = sum_e v[c,f,e] v[c,g,e]; the FM linear
    term folds into W1's top half, b1 folds into a constant-1 row of fm.
  * W1 runs "flipped": stationary = activation blocks [128 feat, 128 batch],
    moving = W1 [128 feat, 512 hid]; h comes out batch-major, the final
    h @ W2 contraction runs on the vector engine (mul + free-dim reduce).
"""

import sys

sys.path.insert(0, "/opt/trn_rl_repo")

import numpy as np
import ml_dtypes

import concourse.bass as bass
import concourse.tile as tile
from concourse import bacc, mybir
from concourse.bass_utils import run_bass_kernel_spmd

NDF, NCF, NCC = 512, 256, 256
EMB, FIELD = 16, 8
B = 2048
NH0 = NDF + 2 * NCC          # 1024
CHANNEL = NH0 - FIELD + 1    # 1017
HID = (NH0 + CHANNEL) // 2   # 1020
NCORES = 8
BC = B // NCORES             # 256 batch rows per core
CT = 121                     # channels per overlapped tile (121 + 7 <= 128)
NT = 9                       # number of channel tiles (9*121 = 1089 >= 1017)

F32 = mybir.dt.float32
BF16 = mybir.dt.bfloat16

_cache = {}


def _sel_plan():
    """Selection-matrix plan for building overlapped tiles xo from x groups.

    Returns list over t of [(mat_idx, g), ...]: xo[:, t] = sum M_i^T x[:, g_i].
    t=0 is the identity on group 0 (handled by a DVE copy, no matmul).
    """
    plan = [[] for _ in range(NT)]
    idx = 0
    for t in range(1, NT):
        base = CT * t
        g0 = base // 128
        plan[t].append((idx, g0))
        idx += 1
        if g0 + 1 < 8 and (base + 127) // 128 > g0:
            plan[t].append((idx, g0 + 1))
            idx += 1
    return plan, idx


def _build(b2_val: float):
    nc = bacc.Bacc()

    plan, nsel = _sel_plan()
    NM = nsel + 7            # selection mats + shift mats S_1..S_7

    warm_d = nc.dram_tensor("warm", [128, 512], BF16, kind="ExternalInput")
    xdc_d = nc.dram_tensor("xdc", [128, 6, BC], BF16, kind="ExternalInput")
    Wda_d = nc.dram_tensor("Wda", [128, 4, NDF], BF16, kind="ExternalInput")
    Wdb_d = nc.dram_tensor("Wdb", [128, 2, NDF], BF16, kind="ExternalInput")
    bdc_d = nc.dram_tensor("bdc", [128, 8], F32, kind="ExternalInput")
    mats_d = nc.dram_tensor("mats", [128, NM, 128], BF16, kind="ExternalInput")
    oneh_d = nc.dram_tensor("oneh", [128, BC], BF16, kind="ExternalInput")
    Gm_d = nc.dram_tensor("Gm", [128, 7, NT, 128], BF16, kind="ExternalInput")
    W1a_d = nc.dram_tensor("W1a", [128, 8, 1024], BF16, kind="ExternalInput")
    W1b_d = nc.dram_tensor("W1b", [128, NT, 1024], BF16, kind="ExternalInput")
    w2b_d = nc.dram_tensor("w2b", [128, 1024], BF16, kind="ExternalInput")
    out_d = nc.dram_tensor("out", [128, 2], F32, kind="ExternalOutput")

    AF = mybir.ActivationFunctionType

    with tile.TileContext(nc) as tc:
        with (
            tc.tile_pool(name="w", bufs=1) as wp,
            tc.tile_pool(name="act", bufs=1) as ap,
            tc.tile_pool(name="pgen", bufs=2, space=bass.MemorySpace.PSUM) as pgen,
            tc.tile_pool(name="pfm", bufs=2, space=bass.MemorySpace.PSUM) as pfm,
            tc.tile_pool(name="pw1", bufs=1, space=bass.MemorySpace.PSUM) as pw1,
        ):
            # ---- weight/input DMAs, one issue per DRAM tensor, serialized
            # on sync in consumption order (parallel queues would make all
            # transfers stream concurrently and starve the front) ----
            warm = wp.tile([128, 512], BF16, tag="warm")
            nc.sync.dma_start(warm[:], warm_d[:])
            xdc = wp.tile([128, 6, BC], BF16, tag="xdc")
            nc.sync.dma_start(xdc[:], xdc_d[:])
            Wda = wp.tile([128, 4, NDF], BF16, tag="Wda")
            nc.sync.dma_start(Wda[:], Wda_d[:])
            bdc = wp.tile([128, 8], F32, tag="bdc")
            nc.sync.dma_start(bdc[:], bdc_d[:])
            Wdb = wp.tile([128, 2, NDF], BF16, tag="Wdb")
            nc.sync.dma_start(Wdb[:], Wdb_d[:])
            mats = wp.tile([128, NM, 128], BF16, tag="mats")
            nc.sync.dma_start(mats[:], mats_d[:])
            oneh = wp.tile([128, BC], BF16, tag="oneh")
            nc.sync.dma_start(oneh[:], oneh_d[:])
            W1a = wp.tile([128, 8, 1024], BF16, tag="W1a")
            nc.sync.dma_start(W1a[:], W1a_d[:])
            Gm = wp.tile([128, 7, NT, 128], BF16, tag="Gm")
            nc.sync.dma_start(Gm[:], Gm_d[:])
            w2b = wp.tile([128, 1024], BF16, tag="w2b")
            nc.sync.dma_start(w2b[:], w2b_d[:])
            W1b = wp.tile([128, NT, 1024], BF16, tag="W1b")
            nc.sync.dma_start(W1b[:], W1b_d[:])

            x = ap.tile([128, 8, BC], BF16, tag="x")
            xo = ap.tile([128, NT, BC], BF16, tag="xo")
            D = ap.tile([128, 7, NT, BC], BF16, tag="D")
            fm = ap.tile([128, NT, BC], BF16, tag="fm")
            h = ap.tile([128, 2, 1024], BF16, tag="h")
            hw0 = ap.tile([128, 1024], BF16, tag="hw0")
            hw1 = ap.tile([128, 1024], BF16, tag="hw1")
            hws = [hw0, hw1]
            sg = ap.tile([128, 2], F32, tag="sg")
            ot = ap.tile([128, 2], F32, tag="ot")
            # ---- PE p-state warmup: run junk matmuls into one PSUM slot
            # from when the first tiny DMA lands until the front weights
            # arrive, so the tensor engine is at full clock for real work ----
            psw = pfm.tile([128, BC], F32, tag="pf")
            for i in range(18):
                nc.tensor.matmul(
                    psw[:], warm[0:128, 0:128], warm[:, 0:BC],
                    start=True, stop=True,
                )

            # ---- front: x = relu([Xd|Xc] @ [Wd|Wc] + b), feature-major ----
            for mt in range(8):
                ps = pfm.tile([128, BC], F32, tag="pf")
                if mt < 4:
                    kts, W, col = [0, 1, 2, 3], Wda, mt * 128
                else:
                    kts, W, col = [0, 1], Wdb, (mt - 4) * 128
                for i, kt in enumerate(kts):
                    xg = kt if mt < 4 else 4 + kt
                    nc.tensor.matmul(
                        ps[:], W[:, kt, col:col + 128], xdc[:, xg, :],
                        start=(i == 0), stop=(i == len(kts) - 1),
                    )
                nc.scalar.activation(
                    out=x[:, mt, :], in_=ps[:], func=AF.Relu,
                    bias=bdc[:, mt:mt + 1], scale=1.0,
                )

            # ---- xo: overlapped 121-feature tiles (PE selection matmuls) ----
            nc.vector.tensor_copy(xo[:, 0, :], x[:, 0, :])
            for t in range(1, NT):
                ps = pfm.tile([128, BC], F32, tag="pf")
                ents = plan[t]
                for i, (mi, g) in enumerate(ents):
                    nc.tensor.matmul(
                        ps[:], mats[:, mi, :], x[:, g, :],
                        start=(i == 0), stop=(i == len(ents) - 1),
                    )
                nc.scalar.activation(
                    out=xo[:, t, :], in_=ps[:], func=AF.Copy, scale=1.0,
                )

            # ---- xs/D/FM software-pipelined rounds (t-major), with the
            # W1 x-part matmuls interleaved to fill tensor idle slots ----
            nsel0 = _sel_plan()[1]
            chunks = [(0, 2), (2, 2), (4, 2), (6, 2), (8, 1)]
            psW = []
            for q in range(4):
                psWq = pw1.tile([128, 512], F32, tag=f"pw{q}", name=f"psW{q}")
                psW.append(psWq)
            # W1 x-part work items in accumulation-safe order per psW tile
            w1x_items = [(bt, kt, n) for kt in range(8)
                         for bt in range(2) for n in range(2)]
            w1x_pos = 0

            def emit_w1x(count):
                nonlocal w1x_pos
                for _ in range(count):
                    if w1x_pos >= len(w1x_items):
                        return
                    bt, kt, n = w1x_items[w1x_pos]
                    nc.tensor.matmul(
                        psW[2 * bt + n][:],
                        x[:, kt, bt * 128:(bt + 1) * 128],
                        W1a[:, kt, n * 512:(n + 1) * 512],
                        start=(kt == 0), stop=False,
                    )
                    w1x_pos += 1

            def emit_fm_w1fm(j0, w):
                # FM accumulation for tiles [j0, j0+w), each followed by its
                # W1 fm-part matmuls (PSUM accumulation is order-independent;
                # only the start/stop flags must bracket the sequence)
                for t in range(j0, j0 + w):
                    ps = pfm.tile([128, BC], F32, tag="pf")
                    for d in range(1, 8):
                        nc.tensor.matmul(
                            ps[:], Gm[:, d - 1, t, :], D[:, d - 1, t, :],
                            start=(d == 1), stop=(d == 7),
                        )
                    if t < NT - 1:
                        nc.scalar.activation(
                            out=fm[:, t, :], in_=ps[:], func=AF.Copy,
                            scale=1.0,
                        )
                    else:
                        # adds the constant-1 row for the b1 fold (row 121)
                        nc.vector.tensor_add(fm[:, t, :], ps[:], oneh[:])

            for k, (j0, w) in enumerate(chunks):
                for d in range(1, 8):
                    ps = pgen.tile([128, 512], F32, tag="pg")
                    nc.tensor.matmul(
                        ps[:, 0:w * BC], mats[:, nsel0 + d - 1, :],
                        xo[:, j0:j0 + w, :], start=True, stop=True,
                    )
                    nc.vector.tensor_mul(
                        D[:, d - 1, j0:j0 + w, :], xo[:, j0:j0 + w, :],
                        ps[:, 0:w * BC],
                    )
                emit_w1x(7)
                if k > 0:
                    emit_fm_w1fm(*chunks[k - 1])
            emit_w1x(len(w1x_items))
            emit_fm_w1fm(*chunks[-1])

            # ---- W1 fm-part + lrelu + W2 (per batch-half, pipelined) ----
            for bt in range(2):
                for kt in range(NT):
                    for n in range(2):
                        nc.tensor.matmul(
                            psW[2 * bt + n][:],
                            fm[:, kt, bt * 128:(bt + 1) * 128],
                            W1b[:, kt, n * 512:(n + 1) * 512],
                            start=False, stop=(kt == NT - 1),
                        )
                for n in range(2):
                    nc.scalar.activation(
                        out=h[:, bt, n * 512:(n + 1) * 512],
                        in_=psW[2 * bt + n][:], func=AF.Lrelu,
                        scale=1.0, alpha=0.01,
                    )
                # W2 on DVE: sum_hid h*w2b
                nc.vector.tensor_mul(hws[bt][:], h[:, bt, :], w2b[:])
                nc.vector.tensor_reduce(
                    sg[:, bt:bt + 1], hws[bt][:],
                    axis=mybir.AxisListType.X, op=mybir.AluOpType.add,
                )
            nc.scalar.activation(
                out=ot[:], in_=sg[:], func=AF.Sigmoid, bias=b2_val, scale=1.0,
            )
            nc.sync.dma_start(out_d[:], ot[:])

    nc.finalize()
    return nc


def _prep_shared(inputs):
    """Host-side weight prep shared across cores (all bf16 on the wire)."""
    bf = ml_dtypes.bfloat16
    Wd = np.asarray(inputs["W_d"], np.float32)
    bd = np.asarray(inputs["b_d"], np.float32)
    Wc = np.asarray(inputs["W_c"], np.float32)
    bc = np.asarray(inputs["b_c"], np.float32)
    v = np.asarray(inputs["v"], np.float32)[0]          # [CHANNEL, FIELD, EMB]
    lin_w = np.asarray(inputs["lin_w"], np.float32)     # [FIELD, 1]
    lin_b = np.asarray(inputs["lin_b"], np.float32)     # [1]
    W1 = np.asarray(inputs["W1"], np.float32)           # [2041, HID]
    b1 = np.asarray(inputs["b1"], np.float32)
    W2 = np.asarray(inputs["W2"], np.float32)           # [HID, 1]

    # Wda/Wdb: stationary front blocks (Wd k-blocks / Wc k-blocks).
    Wda = np.zeros((128, 4, NDF), np.float32)
    for kt in range(4):
        Wda[:, kt, :] = Wd[kt * 128:(kt + 1) * 128, :]
    Wdb = np.zeros((128, 2, NDF), np.float32)
    for kt in range(2):
        Wdb[:, kt, :] = Wc[kt * 128:(kt + 1) * 128, :]
    bdc = np.concatenate([bd, bc]).reshape(8, 128).T.copy()  # [128, 8]

    # selection + shift matrices
    plan, nsel = _sel_plan()
    NM = nsel + 7
    mats = np.zeros((128, NM, 128), np.float32)
    for t in range(1, NT):
        base = CT * t
        for (mi, g) in plan[t]:
            for p in range(128):
                f = base + p
                if f >= NH0:
                    continue
                if f // 128 == g:
                    mats[f - 128 * g, mi, p] = 1.0
    for d in range(1, 8):
        for p in range(128 - d):
            mats[p + d, nsel + d - 1, p] = 1.0

    # banded FM weights on overlapped tiles:
    # Gm[p, d-1, t, m] = G[c=121t+m, f=p-m, f+d], 0<=f<=7-d, m<=120, c<CHANNEL
    G = np.einsum("cfe,cge->cfg", v, v)                 # [CHANNEL, 8, 8]
    Gm = np.zeros((128, 7, NT, 128), np.float32)
    for d in range(1, 8):
        for t in range(NT):
            for m in range(min(CT, CHANNEL - CT * t)):
                c = CT * t + m
                for f in range(0, 8 - d):
                    Gm[m + f, d - 1, t, m] = G[c, f, f + d]

    # fold the FM linear term (x_fm @ lin_w + lin_b) into W1's top half / b1
    W1a = W1[:NH0].copy()                               # [1024, HID]
    W1bfull = W1[NH0:]                                  # [CHANNEL, HID]
    for f in range(FIELD):
        W1a[f:f + CHANNEL, :] += lin_w[f, 0] * W1bfull
    b1e = b1 + lin_b[0] * W1bfull.sum(0)

    W1a_p = np.zeros((128, 8, 1024), np.float32)
    for kt in range(8):
        W1a_p[:, kt, :HID] = W1a[kt * 128:(kt + 1) * 128, :]
    # W1b on overlapped-tile rows; bias row at (t=8, p=121)
    W1b_p = np.zeros((128, NT, 1024), np.float32)
    for t in range(NT):
        for p in range(min(CT, max(0, CHANNEL - CT * t))):
            W1b_p[p, t, :HID] = W1bfull[CT * t + p, :]
    W1b_p[121, 8, :HID] = b1e

    w2b = np.zeros((128, 1024), np.float32)
    w2b[:, :HID] = W2[:, 0][None, :]

    oneh = np.zeros((128, BC), np.float32)
    oneh[121, :] = 1.0

    shared = {
        "warm": np.ones((128, 512), np.float32).astype(bf),
        "Wda": Wda.astype(bf),
        "Wdb": Wdb.astype(bf),
        "bdc": np.ascontiguousarray(bdc),
        "mats": mats.astype(bf),
        "oneh": oneh.astype(bf),
        "Gm": Gm.astype(bf),
        "W1a": W1a_p.astype(bf),
        "W1b": W1b_p.astype(bf),
        "w2b": w2b.astype(bf),
    }
    b2_val = float(np.asarray(inputs["b2"], np.float32)[0])
    return shared, b2_val


def _make_in_maps(inputs, shared):
    dx = np.asarray(inputs["discrete_x"], np.float32)   # [B, NDF]
    cx = np.asarray(inputs["continous_x"], np.float32)  # [B, NCF]
    bf = ml_dtypes.bfloat16
    in_maps = []
    for i in range(NCORES):
        dxi = dx[i * BC:(i + 1) * BC]                   # [BC, 512]
        cxi = cx[i * BC:(i + 1) * BC]                   # [BC, 256]
        xdc = np.empty((128, 6, BC), np.float32)
        for kt in range(4):
            xdc[:, kt, :] = dxi[:, kt * 128:(kt + 1) * 128].T
        for kt in range(2):
            xdc[:, 4 + kt, :] = cxi[:, kt * 128:(kt + 1) * 128].T
        m = dict(shared)
        m["xdc"] = xdc.astype(bf)
        in_maps.append(m)
    return in_maps


def kernel(**inputs) -> np.ndarray:
    shared, b2_val = _prep_shared(inputs)

    if "nc" not in _cache or _cache.get("b2") != b2_val:
        _cache["nc"] = _build(b2_val)
        _cache["b2"] = b2_val
    nc = _cache["nc"]

    in_maps = _make_in_maps(inputs, shared)
    res = run_bass_kernel_spmd(nc, in_maps, core_ids=list(range(NCORES)))
    out = np.empty((B, 1), np.float32)
    for i in range(NCORES):
        o = res.results[i]["out"]                       # [128, 2]
        out[i * BC:i * BC + 128, 0] = o[:, 0]
        out[i * BC + 128:(i + 1) * BC, 0] = o[:, 1]
    return out
